# revision 1
# baseline (speedup 1.0000x reference)
"""Trainium2 Bass kernel for nn_MultiHeadAttention_22419729285517.

Reference computation (softmax-free multi-head attention):
    qkv = x @ w_qkv + b_qkv            # [B,N,3C] -> q,k,v  [B,H,N,D]
    attn = (q @ k^T) / sqrt(D)         # [B,H,N,N]  (NO softmax)
    out  = attn @ v                    # [B,H,N,D]
    out  = concat_heads(out) @ w_proj + b_proj

Because there is no softmax, attention is associative:
    (q @ k^T) @ v = q @ (k^T @ v)
so the N x N attention matrix never needs to exist.  Per head,
kv_h = k_h^T @ v_h is just [D,D] = [64,64].  Folding the output
projection in as well, the whole computation per batch b becomes

    out_b = q_b @ R_b + b_proj
    R_b[h*D+i, :] = sum_j kv_{b,h}[i,j] * w_proj[h*D+j, :]

Sharding (8 cores): sequence-parallel.  Core c owns rows
[s*1024,(s+1)*1024) of batch b, where b = c//4, s = c%4.  Each core:
  1. k,v = x_c @ w_kv                      (local rows, all heads)
  2. vk_h(partial) = v_h^T @ k_h           (= kv_h^T, partial over rows;
     heads processed two-at-a-time as 128x128 block matmuls)
  3. AllReduce vk over the 4 cores of the same batch
  4. q^T projection                        (overlaps the AllReduce)
  5. R rows = blockdiag(vk pair) @ w_proj row-pairs
  6. outT = R^T-as-lhsT @ q^T   -> [768, 1024] f32 (transposed; host
     transposes back — keeps every matmul at the max 512 moving dim)
The 1/sqrt(D) = 0.125 scale is folded into w_q on the host (exact in
bf16: power of two).  b_proj is added on the host (free, general).
All matmuls run in bf16 with fp32 PSUM accumulation (fp32 matmul is 2x
slower on PE); host pre-casts inputs to bf16.
"""

import numpy as np
import ml_dtypes

import concourse.bass as bass
import concourse.mybir as mybir
from concourse import bacc, tile
from concourse import bass_utils

BF16 = mybir.dt.bfloat16
F32 = mybir.dt.float32

B, N, C = 2, 4096, 768
H, D = 12, 64
NCORES = 8
ROWS = (B * N) // NCORES  # 1024 rows per core
KT = C // 128  # 6 contraction tiles of 128
MT = ROWS // 128  # 8 row tiles per core
NP_ = H // 2  # 6 head pairs
NB = ml_dtypes.bfloat16


def _emit_body(nc, tc, pools, tensors, rep, use_collective=True):
    """One full computation pass. rep: unique suffix for tile names."""
    wpool, apool, psum, psum_vk, opool, dram = pools
    x_in, xT, wk, wv, wq, wproj, out = tensors
    replica_groups = [[0, 1, 2, 3], [4, 5, 6, 7]]

    # ---- load inputs to SBUF (x first: the Gram phase needs it) ----
    x_sb, xT_sb, wk_sb, wv_sb, wq_sb, wproj_sb = [], [], [], [], [], []
    for m in range(MT):
        xm = apool.tile([128, C], BF16, name=f"x_m{m}_{rep}", tag=f"x_m{m}", bufs=2)
        if m == 0:
            # split so the first G matmul's operands arrive sooner
            nc.sync.dma_start(xm[:, :512], x_in[0:128, 0:512])
            nc.sync.dma_start(xm[:, 512:], x_in[0:128, 512:C])
        else:
            nc.sync.dma_start(xm[:], x_in[m * 128 : (m + 1) * 128, :])
        x_sb.append(xm)
    for kt in range(KT):
        wk_t = wpool.tile([128, C], BF16, name=f"wk_t{kt}_{rep}", tag=f"wk_t{kt}")
        nc.sync.dma_start(wk_t[:], wk[kt * 128 : (kt + 1) * 128, :])
        wk_sb.append(wk_t)
        x_t = apool.tile(
            [128, ROWS], BF16, name=f"x_t{kt}_{rep}", tag=f"x_t{kt}", bufs=2
        )
        nc.sync.dma_start(x_t[:], xT[kt * 128 : (kt + 1) * 128, :])
        xT_sb.append(x_t)
    for kt in range(KT):
        wv_t = wpool.tile([128, C], BF16, name=f"wv_t{kt}_{rep}", tag=f"wv_t{kt}")
        nc.sync.dma_start(wv_t[:], wv[kt * 128 : (kt + 1) * 128, :])
        wv_sb.append(wv_t)
        wq_t = wpool.tile([128, C], BF16, name=f"wq_t{kt}_{rep}", tag=f"wq_t{kt}")
        nc.sync.dma_start(wq_t[:], wq[kt * 128 : (kt + 1) * 128, :])
        wq_sb.append(wq_t)
    for p in range(NP_):
        wp_t = wpool.tile([128, C], BF16, name=f"wp_t{p}_{rep}", tag=f"wp_t{p}")
        nc.sync.dma_start(wp_t[:], wproj[p * 128 : (p + 1) * 128, :])
        wproj_sb.append(wp_t)

    # ---- phase 1: local Gram matrix G = x_c^T x_c  [768, 768] bf16 ----
    # k,v are only ever used through vk_h = v_h^T k_h = Wv_h^T G Wk_h, so
    # k,v themselves are never materialized.  G is symmetric (and exactly
    # so after rounding: G[a,b] and G[b,a] share the same f32 sum order),
    # which lets G tiles serve directly as their own transposed lhsT.
    G_sb = [
        apool.tile([128, C], BF16, name=f"g_t{it}_{rep}", tag=f"g_t{it}")
        for it in range(KT)
    ]
    for it in range(KT):
        ps = psum.tile([128, C], F32, name="ps_g", tag="mm")
        for m in range(MT):  # stationary x[m][:,it] reused across j chunks
            for j0, jn in ((0, 512), (512, 256)):
                nc.tensor.matmul(
                    ps[:, j0 : j0 + jn],
                    x_sb[m][:, it * 128 : (it + 1) * 128],
                    x_sb[m][:, j0 : j0 + jn],
                    start=(m == 0),
                    stop=(m == MT - 1),
                )
        if it % 2 == 1:
            nc.vector.tensor_copy(G_sb[it][:], ps[:])
        else:
            nc.scalar.copy(G_sb[it][:], ps[:])

    # ---- phase 1b: GWk = G @ w_k  [768, 768] bf16 ----
    GWk_sb = [
        apool.tile([128, C], BF16, name=f"gwk_t{at}_{rep}", tag=f"gwk_t{at}")
        for at in range(KT)
    ]
    for at in range(KT):
        ps = psum.tile([128, C], F32, name="ps_gwk", tag="mm")
        for bt in range(KT):  # lhsT = G[bt][:, at] == G^T block by symmetry
            for i0, inn in ((0, 512), (512, 256)):
                nc.tensor.matmul(
                    ps[:, i0 : i0 + inn],
                    G_sb[bt][:, at * 128 : (at + 1) * 128],
                    wk_sb[bt][:, i0 : i0 + inn],
                    start=(bt == 0),
                    stop=(bt == KT - 1),
                )
        if at % 2 == 1:
            nc.vector.tensor_copy(GWk_sb[at][:], ps[:])
        else:
            nc.scalar.copy(GWk_sb[at][:], ps[:])

    # ---- phase 2: vk pair-blocks = Wv-pair^T @ GWk-pair-cols ----
    # pair p = heads (2p, 2p+1): psum block [128, 128] whose diag 64x64
    # sub-blocks are vk_{2p} and vk_{2p+1}; off-diag cross-head garbage
    # is never copied out (strided diag extraction below)
    ps_vk = [
        psum_vk.tile([128, 384], F32, name=f"ps_vk{g}", tag=f"vk{g}")
        for g in range(2)
    ]
    for p in range(NP_):
        ps = ps_vk[p // 3]
        col = (p % 3) * 128
        for at in range(KT):
            nc.tensor.matmul(
                ps[:, col : col + 128],
                wv_sb[at][:, p * 128 : (p + 1) * 128],  # Wv pair cols
                GWk_sb[at][:, p * 128 : (p + 1) * 128],  # GWk pair cols
                start=(at == 0),
                stop=(at == KT - 1),
            )
    # vk_sb [128, 384] bf16: col block p holds the pair's diag 64x64
    # blocks only (partitions 0:64 = vk_{2p}, 64:128 = vk_{2p+1}),
    # extracted from the psum pair-blocks with strided casting copies —
    # the off-diag cross-head products are never copied out
    vk_sb = apool.tile([128, 384], BF16, name=f"vk_sb_{rep}", tag="vk_sb")
    for g in range(2):
        ps3 = ps_vk[g].rearrange("p (pr s) -> p pr s", s=128)
        dst = vk_sb[:, g * 192 : (g + 1) * 192].rearrange(
            "p (pr d) -> p pr d", d=64
        )
        nc.vector.tensor_copy(dst[0:64], ps3[0:64, :, 0:64])
        nc.vector.tensor_copy(dst[64:128], ps3[64:128, :, 64:128])

    # ---- phase 3: AllReduce vk (bf16, 96 KB) over the 4-core group ----
    vkr = apool.tile([128, 384], BF16, name=f"vkr_{rep}", tag="vkr")
    if use_collective:
        cc_in = dram.tile([128, 384], BF16, name=f"cc_in_{rep}", tag="cc_in")
        cc_out = dram.tile([128, 384], BF16, name=f"cc_out_{rep}", tag="cc_out")
        # scalar-engine DMA queue: keeps the collective's bounce hops off
        # the sync queue, which is busy draining the big input loads
        nc.scalar.dma_start(cc_in[:], vk_sb[:])
        nc.gpsimd.collective_compute(
            "AllReduce",
            mybir.AluOpType.add,
            replica_groups=replica_groups,
            ins=[cc_in.opt()],
            outs=[cc_out.opt()],
        )
        nc.scalar.dma_start(vkr[:], cc_out[:])
    else:
        nc.vector.tensor_copy(vkr[:], vk_sb[:])

    # ---- phase 4: q^T -> qT_sb[t] [128,1024] (overlaps the AllReduce) ----
    qT_sb = [
        apool.tile([128, ROWS], BF16, name=f"q_t{t}_{rep}", tag=f"q_t{t}")
        for t in range(KT)
    ]
    for t in range(KT):
        ps = psum.tile([128, ROWS], F32, name="ps_q", tag="mm")
        for kt in range(KT):  # stationary wq[kt][:,t] reused across mc
            for mc in range(ROWS // 512):
                nc.tensor.matmul(
                    ps[:, mc * 512 : (mc + 1) * 512],
                    wq_sb[kt][:, t * 128 : (t + 1) * 128],
                    xT_sb[kt][:, mc * 512 : (mc + 1) * 512],
                    start=(kt == 0),
                    stop=(kt == KT - 1),
                )
        if t % 2 == 0:
            nc.vector.tensor_copy(qT_sb[t][:], ps[:])
        else:
            nc.scalar.copy(qT_sb[t][:], ps[:])

    # ---- phase 5: R row-pairs = blockdiag(vk pair) @ w_proj row-pair ----
    R_sb = [
        apool.tile([128, C], BF16, name=f"r_t{p}_{rep}", tag=f"r_t{p}")
        for p in range(NP_)
    ]
    for p in range(NP_):
        ps = psum.tile([128, C], F32, name="ps_r", tag="mm")
        for n0, nn in ((0, 512), (512, 256)):
            # even head of the pair: partitions 0:64 of psum
            nc.tensor.matmul(
                ps[0:64, n0 : n0 + nn],
                vkr[0:64, p * 64 : (p + 1) * 64],
                wproj_sb[p][0:64, n0 : n0 + nn],
                start=True,
                stop=True,
            )
            # odd head: partitions 64:128 (lhsT/rhs/out all base 64)
            nc.tensor.matmul(
                ps[64:128, n0 : n0 + nn],
                vkr[64:128, p * 64 : (p + 1) * 64],
                wproj_sb[p][64:128, n0 : n0 + nn],
                start=True,
                stop=True,
            )
        if p % 2 == 1:
            nc.vector.tensor_copy(R_sb[p][:], ps[:])
        else:
            nc.scalar.copy(R_sb[p][:], ps[:])

    # ---- phase 6: outT = R-as-lhsT @ qT  -> [768, 1024] (transposed) ----
    for nt in range(KT):  # 6 output col tiles of 128 (C dim)
        o_t = opool.tile([128, ROWS], F32, name="o_t", tag="o_t")
        ps = psum.tile([128, ROWS], F32, name="ps_o", tag="mm")
        for dt in range(KT):  # stationary R[dt][:,nt] reused across mc
            for mc in range(ROWS // 512):
                nc.tensor.matmul(
                    ps[:, mc * 512 : (mc + 1) * 512],
                    R_sb[dt][:, nt * 128 : (nt + 1) * 128],
                    qT_sb[dt][:, mc * 512 : (mc + 1) * 512],
                    start=(dt == 0),
                    stop=(dt == KT - 1),
                )
        for mc in range(2):
            sl = slice(mc * 512, (mc + 1) * 512)
            if (nt + mc) % 2 == 0:
                nc.vector.tensor_copy(o_t[:, sl], ps[:, sl])
            else:
                nc.scalar.copy(o_t[:, sl], ps[:, sl])
            nc.gpsimd.dma_start(out[nt * 128 : (nt + 1) * 128, sl], o_t[:, sl])


def _build_kernel(repeat=1, use_collective=True, num_devices=NCORES):
    nc = bacc.Bacc(
        "TRN2", target_bir_lowering=False, debug=False, num_devices=num_devices
    )

    x_in = nc.dram_tensor("x", [ROWS, C], BF16, kind="ExternalInput")
    xT = nc.dram_tensor("xT", [C, ROWS], BF16, kind="ExternalInput")
    wk = nc.dram_tensor("wk", [C, C], BF16, kind="ExternalInput")
    wv = nc.dram_tensor("wv", [C, C], BF16, kind="ExternalInput")
    wq = nc.dram_tensor("wq", [C, C], BF16, kind="ExternalInput")
    wproj = nc.dram_tensor("wproj", [C, C], BF16, kind="ExternalInput")
    # transposed output [C, ROWS]; host transposes back
    out = nc.dram_tensor("out", [C, ROWS], F32, kind="ExternalOutput")

    with tile.TileContext(nc) as tc:
        with (
            tc.tile_pool(name="weights", bufs=2) as wpool,
            tc.tile_pool(name="acts", bufs=1) as apool,
            tc.tile_pool(name="psum", bufs=3, space="PSUM") as psum,
            tc.tile_pool(name="psum_vk", bufs=1, space="PSUM") as psum_vk,
            tc.tile_pool(name="outp", bufs=3) as opool,
            tc.tile_pool(name="dram", bufs=2, space="DRAM") as dram,
        ):
            pools = (wpool, apool, psum, psum_vk, opool, dram)
            tensors = (x_in, xT, wk, wv, wq, wproj, out)
            for rep in range(repeat):
                _emit_body(nc, tc, pools, tensors, rep, use_collective)

    nc.compile()
    return nc


_NC_CACHE = None


def _get_nc():
    global _NC_CACHE
    if _NC_CACHE is None:
        _NC_CACHE = _build_kernel()
    return _NC_CACHE


def _numpy_fallback(x, w_qkv, b_qkv, w_proj, b_proj):
    qkv = (x @ w_qkv + b_qkv).reshape(B, N, 3, H, D).transpose(2, 0, 3, 1, 4)
    q, k, v = qkv[0], qkv[1], qkv[2]
    out = np.zeros((B, N, C), np.float32)
    for b in range(B):
        for h in range(H):
            kv = k[b, h].T @ v[b, h]
            out[b, :, h * D : (h + 1) * D] = (q[b, h] / np.sqrt(D)) @ kv
    return out @ w_proj + b_proj


def _make_in_maps(x, w_qkv, w_proj):
    wq_np = np.ascontiguousarray((w_qkv[:, :C] * 0.125)).astype(NB)
    wk_np = np.ascontiguousarray(w_qkv[:, C : 2 * C]).astype(NB)
    wv_np = np.ascontiguousarray(w_qkv[:, 2 * C :]).astype(NB)
    wproj_np = np.ascontiguousarray(w_proj).astype(NB)
    x2 = np.asarray(x, np.float32).reshape(B * N, C)
    in_maps = []
    for c in range(NCORES):
        xc = x2[c * ROWS : (c + 1) * ROWS, :]
        x_np = np.ascontiguousarray(xc).astype(NB)
        xT_np = np.ascontiguousarray(xc.T).astype(NB)
        in_maps.append(
            {
                "x": x_np,
                "xT": xT_np,
                "wk": wk_np,
                "wv": wv_np,
                "wq": wq_np,
                "wproj": wproj_np,
            }
        )
    return in_maps


def kernel(x, w_qkv, b_qkv, w_proj, b_proj, **_kwargs):
    x = np.ascontiguousarray(x, dtype=np.float32)
    w_qkv = np.asarray(w_qkv, dtype=np.float32)
    b_qkv = np.asarray(b_qkv, dtype=np.float32)
    w_proj = np.asarray(w_proj, dtype=np.float32)
    b_proj = np.asarray(b_proj, dtype=np.float32)

    if np.abs(b_qkv).max() != 0:
        # problem spec fills b_qkv with zeros; keep a general fallback
        return _numpy_fallback(x, w_qkv, b_qkv, w_proj, b_proj).astype(np.float32)

    in_maps = _make_in_maps(x, w_qkv, w_proj)
    nc = _get_nc()
    res = bass_utils.run_bass_kernel_spmd(
        nc, in_maps, core_ids=list(range(NCORES))
    )
    out = np.empty((B * N, C), np.float32)
    for c in range(NCORES):
        out[c * ROWS : (c + 1) * ROWS, :] = res.results[c]["out"].T
    out = out.reshape(B, N, C)
    if np.abs(b_proj).max() != 0:
        out = out + b_proj
    return out.astype(np.float32)


if __name__ == "__main__":
    rng = np.random.default_rng(0)
    inputs = {
        "x": rng.standard_normal((B, N, C), dtype=np.float32),
        "w_qkv": (rng.standard_normal((C, 3 * C)) * 0.02).astype(np.float32),
        "b_qkv": np.zeros((3 * C,), np.float32),
        "w_proj": (rng.standard_normal((C, C)) * 0.02).astype(np.float32),
        "b_proj": np.zeros((C,), np.float32),
    }
    got = kernel(**inputs)
    want = _numpy_fallback(**inputs)
    err = np.linalg.norm(got - want) / np.linalg.norm(want)
    print("rel l2 err vs numpy:", err)



# revision 2
# speedup vs baseline: 1.0560x; 1.0560x over previous
"""Trainium2 Bass kernel for nn_MultiHeadAttention_22419729285517.

Softmax-free attention is associative, so the N x N attention matrix
never exists: per head, kv_h = k_h^T v_h is [64, 64], and with
G = x_c^T x_c (the local Gram matrix) the whole kv path collapses to
vk_h = Wv_h^T G Wk_h.  Sequence-parallel over 8 cores (1024 rows
each); the two batches form two 4-core groups that AllReduce the
96 KB vk, overlapped by the q projection.  The 1/sqrt(D) scale is
folded into Wq on the host; all matmuls run bf16 with f32 PSUM.

On top of that baseline structure:

  * Triangle-Gram: G is symmetric, so only the 21 lower-triangle
    128x128 blocks are computed (21.5K streaming cycles vs 36.9K); the
    15 upper blocks are PE transpose-mode copies (128 cyc each) into
    per-block tiles (finest-grained deps).  Bit-identical to computing
    them, so G stays exactly symmetric.
  * bf16 output (halves store traffic; host upcasts to f32).
  * Software-pipelined repeat loop: rep r's post-AllReduce phases are
    emitted after rep r+1's pre phases, so in steady state the
    collective hides behind a full rep of independent PE work
    (identical instruction stream for repeat=1).
"""

import numpy as np
import ml_dtypes

import concourse.bass as bass
import concourse.mybir as mybir
from concourse import bacc, tile
from concourse import bass_utils

BF16 = mybir.dt.bfloat16
F32 = mybir.dt.float32

B, N, C = 2, 4096, 768
H, D = 12, 64
NCORES = 8
ROWS = (B * N) // NCORES  # 1024 rows per core
KT = C // 128  # 6 contraction tiles of 128
MT = ROWS // 128  # 8 row tiles per core
NP_ = H // 2  # 6 head pairs
NB = ml_dtypes.bfloat16


def _emit_pre(nc, tc, pools, tensors, rep, use_collective=True):
    """Loads + Gram + GWk + vk + AllReduce trigger + q projection.

    Returns the (vkr, qT_sb) handles the post-phase needs.  Split from
    _emit_post so the repeat loop can software-pipeline: rep r's post
    phases are emitted after rep r+1's pre phases, which parks ~37 us of
    independent PE work between the AllReduce trigger and its first
    consumer (double-buffered vkr/vk_sb/qT make that legal).
    """
    wpool, apool, psum, psum_vk, opool, dram = pools
    x_in, xT, wk, wv, wq, wproj, ident_in, out = tensors
    replica_groups = [[0, 1, 2, 3], [4, 5, 6, 7]]

    # ---- load inputs to SBUF (x first: the Gram phase needs it) ----
    x_sb, xT_sb, wk_sb, wv_sb, wq_sb, wproj_sb = [], [], [], [], [], []
    ident = wpool.tile([128, 128], BF16, name=f"ident_{rep}", tag="ident")
    nc.scalar.dma_start(ident[:], ident_in[:])
    for m in range(MT):
        xm = apool.tile([128, C], BF16, name=f"x_m{m}_{rep}", tag=f"x_m{m}", bufs=2)
        if m == 0:
            # split so the first G matmul's operands arrive sooner
            nc.sync.dma_start(xm[:, :512], x_in[0:128, 0:512])
            nc.sync.dma_start(xm[:, 512:], x_in[0:128, 512:C])
        else:
            nc.sync.dma_start(xm[:], x_in[m * 128 : (m + 1) * 128, :])
        x_sb.append(xm)
    for kt in range(KT):
        wk_t = wpool.tile([128, C], BF16, name=f"wk_t{kt}_{rep}", tag=f"wk_t{kt}")
        nc.scalar.dma_start(wk_t[:], wk[kt * 128 : (kt + 1) * 128, :])
        wk_sb.append(wk_t)
        x_t = apool.tile(
            [128, ROWS], BF16, name=f"x_t{kt}_{rep}", tag=f"x_t{kt}", bufs=2
        )
        nc.sync.dma_start(x_t[:], xT[kt * 128 : (kt + 1) * 128, :])
        xT_sb.append(x_t)
    for kt in range(KT):
        wv_t = wpool.tile([128, C], BF16, name=f"wv_t{kt}_{rep}", tag=f"wv_t{kt}")
        nc.scalar.dma_start(wv_t[:], wv[kt * 128 : (kt + 1) * 128, :])
        wv_sb.append(wv_t)
        wq_t = wpool.tile([128, C], BF16, name=f"wq_t{kt}_{rep}", tag=f"wq_t{kt}")
        nc.scalar.dma_start(wq_t[:], wq[kt * 128 : (kt + 1) * 128, :])
        wq_sb.append(wq_t)
    for p in range(NP_):
        wp_t = wpool.tile([128, C], BF16, name=f"wp_t{p}_{rep}", tag=f"wp_t{p}")
        nc.scalar.dma_start(wp_t[:], wproj[p * 128 : (p + 1) * 128, :])
        wproj_sb.append(wp_t)

    # ---- phase 1: local Gram matrix G = x_c^T x_c  [768, 768] bf16 ----
    # k,v are only ever used through vk_h = v_h^T k_h = Wv_h^T G Wk_h, so
    # k,v themselves are never materialized.  G is symmetric, so only the
    # lower-triangle blocks are computed; the upper blocks are PE
    # transpose-mode copies of their mirror (bit-identical, so the
    # transposed tiles serve exactly as G^T blocks for GWk's lhsT).
    G_sb = [
        apool.tile([128, C], BF16, name=f"g_t{it}_{rep}", tag=f"g_t{it}")
        for it in range(KT)
    ]
    # GU[(c, it)] = G block [c-rows, it-cols] (upper, c < it), transposed
    # from strip it's block at cols c — per-block tiles for finest deps
    GU = {}
    for it in range(KT):
        W = (it + 1) * 128  # strip covers blocks at or below the diagonal
        ps = psum.tile([128, W], F32, name="ps_g", tag="mm")
        for m in range(MT):  # stationary x[m][:,it] reused across j chunks
            for j0 in range(0, W, 512):
                jn = min(512, W - j0)
                nc.tensor.matmul(
                    ps[:, j0 : j0 + jn],
                    x_sb[m][:, it * 128 : (it + 1) * 128],
                    x_sb[m][:, j0 : j0 + jn],
                    start=(m == 0),
                    stop=(m == MT - 1),
                )
        if it % 2 == 1:
            nc.vector.tensor_copy(G_sb[it][:, :W], ps[:])
        else:
            nc.scalar.copy(G_sb[it][:, :W], ps[:])
        for c in range(it):
            pst = psum.tile([128, 128], BF16, name="ps_tr", tag="mm")
            nc.tensor.transpose(pst[:], G_sb[it][:, c * 128 : (c + 1) * 128], ident)
            gu = apool.tile(
                [128, 128], BF16, name=f"gu_{c}_{it}_{rep}", tag=f"gu_{c}_{it}"
            )
            if (c + it) % 2 == 1:
                nc.vector.tensor_copy(gu[:], pst[:])
            else:
                nc.scalar.copy(gu[:], pst[:])
            GU[(c, it)] = gu

    def g_lhsT(bt, at):
        """G^T block for contraction strip bt, output strip at."""
        if bt >= at:  # at or below the diagonal: computed directly
            return G_sb[bt][:, at * 128 : (at + 1) * 128]
        return GU[(bt, at)][:]

    # ---- phase 1b: GWk = G @ w_k  [768, 768] bf16 ----
    GWk_sb = [
        apool.tile([128, C], BF16, name=f"gwk_t{at}_{rep}", tag=f"gwk_t{at}")
        for at in range(KT)
    ]
    for at in range(KT):
        ps = psum.tile([128, C], F32, name="ps_gwk", tag="mm")
        for bt in range(KT):
            for i0, inn in ((0, 512), (512, 256)):
                nc.tensor.matmul(
                    ps[:, i0 : i0 + inn],
                    g_lhsT(bt, at),
                    wk_sb[bt][:, i0 : i0 + inn],
                    start=(bt == 0),
                    stop=(bt == KT - 1),
                )
        if at % 2 == 1:
            nc.vector.tensor_copy(GWk_sb[at][:], ps[:])
        else:
            nc.scalar.copy(GWk_sb[at][:], ps[:])

    # ---- phase 2: vk pair-blocks = Wv-pair^T @ GWk-pair-cols ----
    # pair p = heads (2p, 2p+1): psum block [128, 128] whose diag 64x64
    # sub-blocks are vk_{2p} and vk_{2p+1}; off-diag cross-head garbage
    # is never copied out (strided diag extraction below)
    ps_vk = [
        psum_vk.tile([128, 384], F32, name=f"ps_vk{g}", tag=f"vk{g}")
        for g in range(2)
    ]
    for p in range(NP_):
        ps = ps_vk[p // 3]
        col = (p % 3) * 128
        for at in range(KT):
            nc.tensor.matmul(
                ps[:, col : col + 128],
                wv_sb[at][:, p * 128 : (p + 1) * 128],  # Wv pair cols
                GWk_sb[at][:, p * 128 : (p + 1) * 128],  # GWk pair cols
                start=(at == 0),
                stop=(at == KT - 1),
            )
    # vk_sb [128, 384] bf16: col block p holds the pair's diag 64x64
    # blocks only (partitions 0:64 = vk_{2p}, 64:128 = vk_{2p+1}),
    # extracted from the psum pair-blocks with strided casting copies —
    # the off-diag cross-head products are never copied out
    vk_sb = apool.tile([128, 384], BF16, name=f"vk_sb_{rep}", tag="vk_sb", bufs=2)
    for g in range(2):
        ps3 = ps_vk[g].rearrange("p (pr s) -> p pr s", s=128)
        dst = vk_sb[:, g * 192 : (g + 1) * 192].rearrange(
            "p (pr d) -> p pr d", d=64
        )
        nc.vector.tensor_copy(dst[0:64], ps3[0:64, :, 0:64])
        nc.vector.tensor_copy(dst[64:128], ps3[64:128, :, 64:128])

    # ---- phase 3: AllReduce vk (bf16, 96 KB) over the 4-core group ----
    vkr = apool.tile([128, 384], BF16, name=f"vkr_{rep}", tag="vkr", bufs=2)
    if use_collective:
        cc_in = dram.tile([128, 384], BF16, name=f"cc_in_{rep}", tag="cc_in")
        cc_out = dram.tile([128, 384], BF16, name=f"cc_out_{rep}", tag="cc_out")
        # scalar-engine DMA queue: keeps the collective's bounce hops off
        # the sync queue, which is busy draining the big input loads
        nc.scalar.dma_start(cc_in[:], vk_sb[:])
        nc.gpsimd.collective_compute(
            "AllReduce",
            mybir.AluOpType.add,
            replica_groups=replica_groups,
            ins=[cc_in.opt()],
            outs=[cc_out.opt()],
        )
        nc.scalar.dma_start(vkr[:], cc_out[:])
    else:
        nc.vector.tensor_copy(vkr[:], vk_sb[:])

    # ---- phase 4: q^T -> qT_sb[t] [128,1024] (overlaps the AllReduce) ----
    qT_sb = [
        apool.tile([128, ROWS], BF16, name=f"q_t{t}_{rep}", tag=f"q_t{t}", bufs=2)
        for t in range(KT)
    ]
    for t in range(KT):
        ps = psum.tile([128, ROWS], F32, name="ps_q", tag="mm")
        for kt in range(KT):  # stationary wq[kt][:,t] reused across mc
            for mc in range(ROWS // 512):
                nc.tensor.matmul(
                    ps[:, mc * 512 : (mc + 1) * 512],
                    wq_sb[kt][:, t * 128 : (t + 1) * 128],
                    xT_sb[kt][:, mc * 512 : (mc + 1) * 512],
                    start=(kt == 0),
                    stop=(kt == KT - 1),
                )
        if t % 2 == 0:
            nc.vector.tensor_copy(qT_sb[t][:], ps[:])
        else:
            nc.scalar.copy(qT_sb[t][:], ps[:])

    return vkr, qT_sb, wproj_sb


def _emit_post(nc, tc, pools, tensors, rep, state):
    """R projection + output GEMM + store (consumes the AllReduce)."""
    wpool, apool, psum, psum_vk, opool, dram = pools
    x_in, xT, wk, wv, wq, wproj, ident_in, out = tensors
    vkr, qT_sb, wproj_sb = state

    # ---- phase 5: R row-pairs = blockdiag(vk pair) @ w_proj row-pair ----
    R_sb = [
        apool.tile([128, C], BF16, name=f"r_t{p}_{rep}", tag=f"r_t{p}")
        for p in range(NP_)
    ]
    for p in range(NP_):
        ps = psum.tile([128, C], F32, name="ps_r", tag="mm")
        for n0, nn in ((0, 512), (512, 256)):
            # even head of the pair: partitions 0:64 of psum
            nc.tensor.matmul(
                ps[0:64, n0 : n0 + nn],
                vkr[0:64, p * 64 : (p + 1) * 64],
                wproj_sb[p][0:64, n0 : n0 + nn],
                start=True,
                stop=True,
            )
            # odd head: partitions 64:128 (lhsT/rhs/out all base 64)
            nc.tensor.matmul(
                ps[64:128, n0 : n0 + nn],
                vkr[64:128, p * 64 : (p + 1) * 64],
                wproj_sb[p][64:128, n0 : n0 + nn],
                start=True,
                stop=True,
            )
        if p % 2 == 1:
            nc.vector.tensor_copy(R_sb[p][:], ps[:])
        else:
            nc.scalar.copy(R_sb[p][:], ps[:])

    # ---- phase 6: outT = R-as-lhsT @ qT  -> [768, 1024] bf16 ----
    for nt in range(KT):  # 6 output col tiles of 128 (C dim)
        o_t = opool.tile([128, ROWS], BF16, name="o_t", tag="o_t")
        ps = psum.tile([128, ROWS], F32, name="ps_o", tag="mm")
        for dt in range(KT):  # stationary R[dt][:,nt] reused across mc
            for mc in range(ROWS // 512):
                nc.tensor.matmul(
                    ps[:, mc * 512 : (mc + 1) * 512],
                    R_sb[dt][:, nt * 128 : (nt + 1) * 128],
                    qT_sb[dt][:, mc * 512 : (mc + 1) * 512],
                    start=(dt == 0),
                    stop=(dt == KT - 1),
                )
        for mc in range(2):
            sl = slice(mc * 512, (mc + 1) * 512)
            if (nt + mc) % 2 == 0:
                nc.vector.tensor_copy(o_t[:, sl], ps[:, sl])
            else:
                nc.scalar.copy(o_t[:, sl], ps[:, sl])
            nc.gpsimd.dma_start(out[nt * 128 : (nt + 1) * 128, sl], o_t[:, sl])


def _build_kernel(repeat=1, use_collective=True, num_devices=NCORES):
    nc = bacc.Bacc(
        "TRN2", target_bir_lowering=False, debug=False, num_devices=num_devices
    )

    x_in = nc.dram_tensor("x", [ROWS, C], BF16, kind="ExternalInput")
    xT = nc.dram_tensor("xT", [C, ROWS], BF16, kind="ExternalInput")
    wk = nc.dram_tensor("wk", [C, C], BF16, kind="ExternalInput")
    wv = nc.dram_tensor("wv", [C, C], BF16, kind="ExternalInput")
    wq = nc.dram_tensor("wq", [C, C], BF16, kind="ExternalInput")
    wproj = nc.dram_tensor("wproj", [C, C], BF16, kind="ExternalInput")
    ident_in = nc.dram_tensor("ident", [128, 128], BF16, kind="ExternalInput")
    # transposed output [C, ROWS] bf16; host transposes back and upcasts
    out = nc.dram_tensor("out", [C, ROWS], BF16, kind="ExternalOutput")

    with tile.TileContext(nc) as tc:
        with (
            tc.tile_pool(name="weights", bufs=2) as wpool,
            tc.tile_pool(name="acts", bufs=1) as apool,
            tc.tile_pool(name="psum", bufs=3, space="PSUM") as psum,
            tc.tile_pool(name="psum_vk", bufs=1, space="PSUM") as psum_vk,
            tc.tile_pool(name="outp", bufs=3) as opool,
            tc.tile_pool(name="dram", bufs=2, space="DRAM") as dram,
        ):
            pools = (wpool, apool, psum, psum_vk, opool, dram)
            tensors = (x_in, xT, wk, wv, wq, wproj, ident_in, out)
            # software-pipelined: post(r) is emitted after pre(r+1), so the
            # AllReduce of rep r has a full rep of PE work to hide behind
            prev = None
            for rep in range(repeat):
                state = _emit_pre(nc, tc, pools, tensors, rep, use_collective)
                if prev is not None:
                    _emit_post(nc, tc, pools, tensors, rep - 1, prev)
                prev = state
            _emit_post(nc, tc, pools, tensors, repeat - 1, prev)

    nc.compile()
    return nc


_NC_CACHE = None


def _get_nc():
    global _NC_CACHE
    if _NC_CACHE is None:
        _NC_CACHE = _build_kernel()
    return _NC_CACHE


def _numpy_fallback(x, w_qkv, b_qkv, w_proj, b_proj):
    qkv = (x @ w_qkv + b_qkv).reshape(B, N, 3, H, D).transpose(2, 0, 3, 1, 4)
    q, k, v = qkv[0], qkv[1], qkv[2]
    out = np.zeros((B, N, C), np.float32)
    for b in range(B):
        for h in range(H):
            kv = k[b, h].T @ v[b, h]
            out[b, :, h * D : (h + 1) * D] = (q[b, h] / np.sqrt(D)) @ kv
    return out @ w_proj + b_proj


def _make_in_maps(x, w_qkv, w_proj):
    wq_np = np.ascontiguousarray((w_qkv[:, :C] * 0.125)).astype(NB)
    wk_np = np.ascontiguousarray(w_qkv[:, C : 2 * C]).astype(NB)
    wv_np = np.ascontiguousarray(w_qkv[:, 2 * C :]).astype(NB)
    wproj_np = np.ascontiguousarray(w_proj).astype(NB)
    ident_np = np.eye(128, dtype=NB)
    x2 = np.asarray(x, np.float32).reshape(B * N, C)
    in_maps = []
    for c in range(NCORES):
        xc = x2[c * ROWS : (c + 1) * ROWS, :]
        x_np = np.ascontiguousarray(xc).astype(NB)
        xT_np = np.ascontiguousarray(xc.T).astype(NB)
        in_maps.append(
            {
                "x": x_np,
                "xT": xT_np,
                "wk": wk_np,
                "wv": wv_np,
                "wq": wq_np,
                "wproj": wproj_np,
                "ident": ident_np,
            }
        )
    return in_maps


def kernel(x, w_qkv, b_qkv, w_proj, b_proj, **_kwargs):
    x = np.ascontiguousarray(x, dtype=np.float32)
    w_qkv = np.asarray(w_qkv, dtype=np.float32)
    b_qkv = np.asarray(b_qkv, dtype=np.float32)
    w_proj = np.asarray(w_proj, dtype=np.float32)
    b_proj = np.asarray(b_proj, dtype=np.float32)

    if np.abs(b_qkv).max() != 0:
        # problem spec fills b_qkv with zeros; keep a general fallback
        return _numpy_fallback(x, w_qkv, b_qkv, w_proj, b_proj).astype(np.float32)

    in_maps = _make_in_maps(x, w_qkv, w_proj)
    nc = _get_nc()
    res = bass_utils.run_bass_kernel_spmd(
        nc, in_maps, core_ids=list(range(NCORES))
    )
    out = np.empty((B * N, C), np.float32)
    for c in range(NCORES):
        out[c * ROWS : (c + 1) * ROWS, :] = res.results[c]["out"].astype(np.float32).T
    out = out.reshape(B, N, C)
    if np.abs(b_proj).max() != 0:
        out = out + b_proj
    return out.astype(np.float32)


if __name__ == "__main__":
    rng = np.random.default_rng(0)
    inputs = {
        "x": rng.standard_normal((B, N, C), dtype=np.float32),
        "w_qkv": (rng.standard_normal((C, 3 * C)) * 0.02).astype(np.float32),
        "b_qkv": np.zeros((3 * C,), np.float32),
        "w_proj": (rng.standard_normal((C, C)) * 0.02).astype(np.float32),
        "b_proj": np.zeros((C,), np.float32),
    }
    got = kernel(**inputs)
    want = _numpy_fallback(**inputs)
    err = np.linalg.norm(got - want) / np.linalg.norm(want)
    print("rel l2 err vs numpy:", err)


# revision 4
# speedup vs baseline: 1.3592x; 1.2871x over previous
"""Trainium2 Bass kernel v4 for nn_MultiHeadAttention_22419729285517.

Gram-folded softmax-free attention, sequence-parallel over 8 cores
(1024 rows each; two 4-core batch groups AllReduce the 96 KB vk).
The N x N attention matrix never exists: vk_h = Wv_h^T G Wk_h with
G = x_c^T x_c.  1/sqrt(D) folds into Wq on the host; bf16 matmuls
with f32 PSUM.  Optimizations over that baseline:

  * Triangle-Gram: G is symmetric, so only the 21 lower-triangle
    128x128 blocks are computed (21.5K streaming cycles vs 36.9K); the
    15 upper blocks are PE transpose-mode copies (128 cyc each) into
    per-block tiles (finest-grained deps).  Bit-identical to computing
    them, so G stays exactly symmetric.
  * bf16 output (halves store traffic; host upcasts to f32).
  * M-trick: out = (x Wq) R = x (Wq R) — the per-row q projection never
    exists; M = Wq R costs 27.6K cycles once per rep vs q's 36.9K, and
    the output GEMM reads xT directly (host ships Wq transposed).
  * Block-diag R: each head-pair's two 64x64 vk blocks sit on the
    diagonal of a persistent zeroed [128,128] tile -> one
    full-contraction stationary per pair (4.6K cycles, was 9.2K).
  * Software-pipelined repeat loop: rep r's post-AllReduce phases are
    emitted after rep r+1's pre phases, hiding the collective behind a
    full rep of PE work (identical stream at repeat=1).
  * Queue layout: collective bounce-in on gpsimd, readback on sync,
    output stores on the ACT HWDGE queue — so the blocking
    collective_compute wait never stalls a queue that later compute
    copies or stores need.
"""

import numpy as np
import ml_dtypes

import concourse.bass as bass
import concourse.mybir as mybir
from concourse import bacc, tile
from concourse import bass_utils

BF16 = mybir.dt.bfloat16
F32 = mybir.dt.float32

B, N, C = 2, 4096, 768
H, D = 12, 64
NCORES = 8
ROWS = (B * N) // NCORES  # 1024 rows per core
KT = C // 128  # 6 contraction tiles of 128
MT = ROWS // 128  # 8 row tiles per core
NP_ = H // 2  # 6 head pairs
NB = ml_dtypes.bfloat16


def _emit_pre(nc, tc, pools, tensors, rep, use_collective=True):
    """Loads + Gram + GWk + vk + AllReduce trigger + q projection.

    Returns the (vkr, qT_sb) handles the post-phase needs.  Split from
    _emit_post so the repeat loop can software-pipeline: rep r's post
    phases are emitted after rep r+1's pre phases, which parks ~37 us of
    independent PE work between the AllReduce trigger and its first
    consumer (double-buffered vkr/vk_sb/qT make that legal).
    """
    wpool, apool, psum, psum_vk, opool, dram = pools
    x_in, xT, wk, wv, wq, wproj, ident_in, out = tensors
    replica_groups = [[0, 1, 2, 3], [4, 5, 6, 7]]

    # ---- load inputs to SBUF (x first: the Gram phase needs it) ----
    x_sb, xT_sb, wk_sb, wv_sb, wq_sb, wproj_sb = [], [], [], [], [], []
    ident = wpool.tile([128, 128], BF16, name=f"ident_{rep}", tag="ident")
    nc.scalar.dma_start(ident[:], ident_in[:])
    for m in range(MT):
        xm = apool.tile([128, C], BF16, name=f"x_m{m}_{rep}", tag=f"x_m{m}", bufs=2)
        if m == 0:
            # split so the first G matmul's operands arrive sooner
            nc.sync.dma_start(xm[:, :512], x_in[0:128, 0:512])
            nc.sync.dma_start(xm[:, 512:], x_in[0:128, 512:C])
        else:
            nc.sync.dma_start(xm[:], x_in[m * 128 : (m + 1) * 128, :])
        x_sb.append(xm)
    for kt in range(KT):
        wk_t = wpool.tile([128, C], BF16, name=f"wk_t{kt}_{rep}", tag=f"wk_t{kt}")
        nc.scalar.dma_start(wk_t[:], wk[kt * 128 : (kt + 1) * 128, :])
        wk_sb.append(wk_t)
        x_t = apool.tile(
            [128, ROWS], BF16, name=f"x_t{kt}_{rep}", tag=f"x_t{kt}", bufs=2
        )
        nc.sync.dma_start(x_t[:], xT[kt * 128 : (kt + 1) * 128, :])
        xT_sb.append(x_t)
    for kt in range(KT):
        wv_t = wpool.tile([128, C], BF16, name=f"wv_t{kt}_{rep}", tag=f"wv_t{kt}")
        nc.scalar.dma_start(wv_t[:], wv[kt * 128 : (kt + 1) * 128, :])
        wv_sb.append(wv_t)
        wq_t = wpool.tile([128, C], BF16, name=f"wq_t{kt}_{rep}", tag=f"wq_t{kt}")
        nc.scalar.dma_start(wq_t[:], wq[kt * 128 : (kt + 1) * 128, :])
        wq_sb.append(wq_t)
    for p in range(NP_):
        wp_t = wpool.tile([128, C], BF16, name=f"wp_t{p}_{rep}", tag=f"wp_t{p}")
        nc.scalar.dma_start(wp_t[:], wproj[p * 128 : (p + 1) * 128, :])
        wproj_sb.append(wp_t)

    # ---- phase 1: local Gram matrix G = x_c^T x_c  [768, 768] bf16 ----
    # k,v are only ever used through vk_h = v_h^T k_h = Wv_h^T G Wk_h, so
    # k,v themselves are never materialized.  G is symmetric, so only the
    # lower-triangle blocks are computed; the upper blocks are PE
    # transpose-mode copies of their mirror (bit-identical, so the
    # transposed tiles serve exactly as G^T blocks for GWk's lhsT).
    G_sb = [
        apool.tile([128, C], BF16, name=f"g_t{it}_{rep}", tag=f"g_t{it}")
        for it in range(KT)
    ]
    # GU[(c, it)] = G block [c-rows, it-cols] (upper, c < it), transposed
    # from strip it's block at cols c — per-block tiles for finest deps
    GU = {}
    for it in range(KT):
        W = (it + 1) * 128  # strip covers blocks at or below the diagonal
        ps = psum.tile([128, W], F32, name="ps_g", tag="mm")
        for m in range(MT):  # stationary x[m][:,it] reused across j chunks
            for j0 in range(0, W, 512):
                jn = min(512, W - j0)
                nc.tensor.matmul(
                    ps[:, j0 : j0 + jn],
                    x_sb[m][:, it * 128 : (it + 1) * 128],
                    x_sb[m][:, j0 : j0 + jn],
                    start=(m == 0),
                    stop=(m == MT - 1),
                )
        if it % 2 == 1:
            nc.vector.tensor_copy(G_sb[it][:, :W], ps[:])
        else:
            nc.scalar.copy(G_sb[it][:, :W], ps[:])
        for c in range(it):
            pst = psum.tile([128, 128], BF16, name="ps_tr", tag="mm")
            nc.tensor.transpose(pst[:], G_sb[it][:, c * 128 : (c + 1) * 128], ident)
            gu = apool.tile(
                [128, 128], BF16, name=f"gu_{c}_{it}_{rep}", tag=f"gu_{c}_{it}"
            )
            if (c + it) % 2 == 1:
                nc.vector.tensor_copy(gu[:], pst[:])
            else:
                nc.scalar.copy(gu[:], pst[:])
            GU[(c, it)] = gu

    def g_lhsT(bt, at):
        """G^T block for contraction strip bt, output strip at."""
        if bt >= at:  # at or below the diagonal: computed directly
            return G_sb[bt][:, at * 128 : (at + 1) * 128]
        return GU[(bt, at)][:]

    # ---- phase 1b: GWk = G @ w_k  [768, 768] bf16 ----
    GWk_sb = [
        apool.tile([128, C], BF16, name=f"gwk_t{at}_{rep}", tag=f"gwk_t{at}")
        for at in range(KT)
    ]
    for at in range(KT):
        ps = psum.tile([128, C], F32, name="ps_gwk", tag="mm")
        for bt in range(KT):
            for i0, inn in ((0, 512), (512, 256)):
                nc.tensor.matmul(
                    ps[:, i0 : i0 + inn],
                    g_lhsT(bt, at),
                    wk_sb[bt][:, i0 : i0 + inn],
                    start=(bt == 0),
                    stop=(bt == KT - 1),
                )
        if at % 2 == 1:
            nc.vector.tensor_copy(GWk_sb[at][:], ps[:])
        else:
            nc.scalar.copy(GWk_sb[at][:], ps[:])

    # ---- phase 2: vk pair-blocks = Wv-pair^T @ GWk-pair-cols ----
    # pair p = heads (2p, 2p+1): psum block [128, 128] whose diag 64x64
    # sub-blocks are vk_{2p} and vk_{2p+1}; off-diag cross-head garbage
    # is never copied out (strided diag extraction below)
    ps_vk = [
        psum_vk.tile([128, 384], F32, name=f"ps_vk{g}", tag=f"vk{g}")
        for g in range(2)
    ]
    for p in range(NP_):
        ps = ps_vk[p // 3]
        col = (p % 3) * 128
        for at in range(KT):
            nc.tensor.matmul(
                ps[:, col : col + 128],
                wv_sb[at][:, p * 128 : (p + 1) * 128],  # Wv pair cols
                GWk_sb[at][:, p * 128 : (p + 1) * 128],  # GWk pair cols
                start=(at == 0),
                stop=(at == KT - 1),
            )
    # vk_sb [128, 384] bf16: col block p holds the pair's diag 64x64
    # blocks only (partitions 0:64 = vk_{2p}, 64:128 = vk_{2p+1}),
    # extracted from the psum pair-blocks with strided casting copies —
    # the off-diag cross-head products are never copied out
    vk_sb = apool.tile([128, 384], BF16, name=f"vk_sb_{rep}", tag="vk_sb", bufs=2)
    for g in range(2):
        ps3 = ps_vk[g].rearrange("p (pr s) -> p pr s", s=128)
        dst = vk_sb[:, g * 192 : (g + 1) * 192].rearrange(
            "p (pr d) -> p pr d", d=64
        )
        nc.vector.tensor_copy(dst[0:64], ps3[0:64, :, 0:64])
        nc.vector.tensor_copy(dst[64:128], ps3[64:128, :, 64:128])

    # ---- phase 3: AllReduce vk (bf16, 96 KB) over the 4-core group ----
    vkr = apool.tile([128, 384], BF16, name=f"vkr_{rep}", tag="vkr", bufs=2)
    if use_collective:
        cc_in = dram.tile([128, 384], BF16, name=f"cc_in_{rep}", tag="cc_in")
        cc_out = dram.tile([128, 384], BF16, name=f"cc_out_{rep}", tag="cc_out")
        # bounce-in on gpsimd: its wait (vk ready) resolves before the
        # out-store DMAs behind it have data; keeping it OFF the ACT queue
        # matters because ACT also runs the post-phase psum copies, and a
        # wait-on-collective there stalls the previous rep's R/M copies
        nc.gpsimd.dma_start(cc_in[:], vk_sb[:])
        nc.gpsimd.collective_compute(
            "AllReduce",
            mybir.AluOpType.add,
            replica_groups=replica_groups,
            ins=[cc_in.opt()],
            outs=[cc_out.opt()],
        )
        # readback on sync: only delays rep r+2's input prefetch (which
        # has a full post-phase of slack), never a compute-copy queue
        nc.sync.dma_start(vkr[:], cc_out[:])
    else:
        nc.vector.tensor_copy(vkr[:], vk_sb[:])

    return vkr, wq_sb, wproj_sb, xT_sb


def _emit_post(nc, tc, pools, tensors, rep, state):
    """R projection + output GEMM + store (consumes the AllReduce)."""
    wpool, apool, psum, psum_vk, opool, dram = pools
    x_in, xT, wk, wv, wq, wproj, ident_in, out = tensors
    vkr, wq_sb, wproj_sb, xT_sb = state

    # ---- phase 5: R row-pairs = blockdiag(vk pair) @ w_proj row-pair ----
    # The pair's two 64x64 vk blocks are copied onto the diagonal of a
    # persistent [128, 128] tile (off-diagonal zeroed once at rep 0 and
    # never written again), so each pair is ONE full-128-contraction
    # stationary instead of two half-width matmuls — halves the R-phase
    # streaming cycles.
    R_sb = [
        apool.tile([128, C], BF16, name=f"r_t{p}_{rep}", tag=f"r_t{p}")
        for p in range(NP_)
    ]
    bd = []
    for p in range(NP_):
        bdp = apool.tile(
            [128, 128], BF16, name=f"bd_{p}_{rep}", tag=f"bd_{p}", bufs=1
        )
        if rep == 0:
            nc.vector.memset(bdp[:], 0.0)
        sl = slice(p * 64, (p + 1) * 64)
        nc.vector.tensor_copy(bdp[0:64, 0:64], vkr[0:64, sl])
        nc.vector.tensor_copy(bdp[64:128, 64:128], vkr[64:128, sl])
        bd.append(bdp)
    for p in range(NP_):
        ps = psum.tile([128, C], F32, name="ps_r", tag="mm")
        for n0, nn in ((0, 512), (512, 256)):
            nc.tensor.matmul(
                ps[:, n0 : n0 + nn],
                bd[p][:],
                wproj_sb[p][:, n0 : n0 + nn],
                start=True,
                stop=True,
            )
        if p % 2 == 1:
            nc.vector.tensor_copy(R_sb[p][:], ps[:])
        else:
            nc.scalar.copy(R_sb[p][:], ps[:])

    # ---- phase 6: M = Wq @ R  [768, 768] bf16 (wq_sb holds Wq^T) ----
    # out = q@R = x@(Wq R) = x@M, so the q projection never exists:
    # M costs 27.6K cycles once vs 36.9K for q, and out reads xT directly
    M_sb = [
        apool.tile([128, C], BF16, name=f"m_t{ct}_{rep}", tag=f"m_t{ct}")
        for ct in range(KT)
    ]
    for ct in range(KT):
        ps = psum.tile([128, C], F32, name="ps_m", tag="mm")
        for dt in range(KT):
            for i0, inn in ((0, 512), (512, 256)):
                nc.tensor.matmul(
                    ps[:, i0 : i0 + inn],
                    wq_sb[dt][:, ct * 128 : (ct + 1) * 128],
                    R_sb[dt][:, i0 : i0 + inn],
                    start=(dt == 0),
                    stop=(dt == KT - 1),
                )
        if ct % 2 == 1:
            nc.vector.tensor_copy(M_sb[ct][:], ps[:])
        else:
            nc.scalar.copy(M_sb[ct][:], ps[:])

    # ---- phase 7: outT = M-as-lhsT @ xT  -> [768, 1024] bf16 ----
    for nt in range(KT):  # 6 output col tiles of 128 (C dim)
        o_t = opool.tile([128, ROWS], BF16, name="o_t", tag="o_t")
        ps = psum.tile([128, ROWS], F32, name="ps_o", tag="mm")
        for ct in range(KT):  # stationary M[ct][:,nt] reused across mc
            for mc in range(ROWS // 512):
                nc.tensor.matmul(
                    ps[:, mc * 512 : (mc + 1) * 512],
                    M_sb[ct][:, nt * 128 : (nt + 1) * 128],
                    xT_sb[ct][:, mc * 512 : (mc + 1) * 512],
                    start=(ct == 0),
                    stop=(ct == KT - 1),
                )
        for mc in range(2):
            sl = slice(mc * 512, (mc + 1) * 512)
            if (nt + mc) % 2 == 0:
                nc.vector.tensor_copy(o_t[:, sl], ps[:, sl])
            else:
                nc.scalar.copy(o_t[:, sl], ps[:, sl])
            # ACT-queue HWDGE store: keeps the out DMAs off gpsimd, whose
            # collective_compute blocks that queue until the AR completes
            nc.scalar.dma_start(out[nt * 128 : (nt + 1) * 128, sl], o_t[:, sl])


def _build_kernel(repeat=1, use_collective=True, num_devices=NCORES):
    nc = bacc.Bacc(
        "TRN2", target_bir_lowering=False, debug=False, num_devices=num_devices
    )

    x_in = nc.dram_tensor("x", [ROWS, C], BF16, kind="ExternalInput")
    xT = nc.dram_tensor("xT", [C, ROWS], BF16, kind="ExternalInput")
    wk = nc.dram_tensor("wk", [C, C], BF16, kind="ExternalInput")
    wv = nc.dram_tensor("wv", [C, C], BF16, kind="ExternalInput")
    wq = nc.dram_tensor("wq", [C, C], BF16, kind="ExternalInput")
    wproj = nc.dram_tensor("wproj", [C, C], BF16, kind="ExternalInput")
    ident_in = nc.dram_tensor("ident", [128, 128], BF16, kind="ExternalInput")
    # transposed output [C, ROWS] bf16; host transposes back and upcasts
    out = nc.dram_tensor("out", [C, ROWS], BF16, kind="ExternalOutput")

    with tile.TileContext(nc) as tc:
        with (
            tc.tile_pool(name="weights", bufs=2) as wpool,
            tc.tile_pool(name="acts", bufs=1) as apool,
            tc.tile_pool(name="psum", bufs=3, space="PSUM") as psum,
            tc.tile_pool(name="psum_vk", bufs=1, space="PSUM") as psum_vk,
            tc.tile_pool(name="outp", bufs=3) as opool,
            tc.tile_pool(name="dram", bufs=2, space="DRAM") as dram,
        ):
            pools = (wpool, apool, psum, psum_vk, opool, dram)
            tensors = (x_in, xT, wk, wv, wq, wproj, ident_in, out)
            # software-pipelined: post(r) is emitted after pre(r+1), so the
            # AllReduce of rep r has a full rep of PE work to hide behind
            prev = None
            for rep in range(repeat):
                state = _emit_pre(nc, tc, pools, tensors, rep, use_collective)
                if prev is not None:
                    _emit_post(nc, tc, pools, tensors, rep - 1, prev)
                prev = state
            _emit_post(nc, tc, pools, tensors, repeat - 1, prev)

    nc.compile()
    return nc


_NC_CACHE = None


def _get_nc():
    global _NC_CACHE
    if _NC_CACHE is None:
        _NC_CACHE = _build_kernel()
    return _NC_CACHE


def _numpy_fallback(x, w_qkv, b_qkv, w_proj, b_proj):
    qkv = (x @ w_qkv + b_qkv).reshape(B, N, 3, H, D).transpose(2, 0, 3, 1, 4)
    q, k, v = qkv[0], qkv[1], qkv[2]
    out = np.zeros((B, N, C), np.float32)
    for b in range(B):
        for h in range(H):
            kv = k[b, h].T @ v[b, h]
            out[b, :, h * D : (h + 1) * D] = (q[b, h] / np.sqrt(D)) @ kv
    return out @ w_proj + b_proj


def _make_in_maps(x, w_qkv, w_proj):
    # shipped TRANSPOSED: the M = Wq@R phase needs Wq with the qkv-out
    # index on partitions (lhsT layout)
    wq_np = np.ascontiguousarray((w_qkv[:, :C] * 0.125).T).astype(NB)
    wk_np = np.ascontiguousarray(w_qkv[:, C : 2 * C]).astype(NB)
    wv_np = np.ascontiguousarray(w_qkv[:, 2 * C :]).astype(NB)
    wproj_np = np.ascontiguousarray(w_proj).astype(NB)
    ident_np = np.eye(128, dtype=NB)
    x2 = np.asarray(x, np.float32).reshape(B * N, C)
    in_maps = []
    for c in range(NCORES):
        xc = x2[c * ROWS : (c + 1) * ROWS, :]
        x_np = np.ascontiguousarray(xc).astype(NB)
        xT_np = np.ascontiguousarray(xc.T).astype(NB)
        in_maps.append(
            {
                "x": x_np,
                "xT": xT_np,
                "wk": wk_np,
                "wv": wv_np,
                "wq": wq_np,
                "wproj": wproj_np,
                "ident": ident_np,
            }
        )
    return in_maps


def kernel(x, w_qkv, b_qkv, w_proj, b_proj, **_kwargs):
    x = np.ascontiguousarray(x, dtype=np.float32)
    w_qkv = np.asarray(w_qkv, dtype=np.float32)
    b_qkv = np.asarray(b_qkv, dtype=np.float32)
    w_proj = np.asarray(w_proj, dtype=np.float32)
    b_proj = np.asarray(b_proj, dtype=np.float32)

    if np.abs(b_qkv).max() != 0:
        # problem spec fills b_qkv with zeros; keep a general fallback
        return _numpy_fallback(x, w_qkv, b_qkv, w_proj, b_proj).astype(np.float32)

    in_maps = _make_in_maps(x, w_qkv, w_proj)
    nc = _get_nc()
    res = bass_utils.run_bass_kernel_spmd(
        nc, in_maps, core_ids=list(range(NCORES))
    )
    out = np.empty((B * N, C), np.float32)
    for c in range(NCORES):
        out[c * ROWS : (c + 1) * ROWS, :] = res.results[c]["out"].astype(np.float32).T
    out = out.reshape(B, N, C)
    if np.abs(b_proj).max() != 0:
        out = out + b_proj
    return out.astype(np.float32)


if __name__ == "__main__":
    rng = np.random.default_rng(0)
    inputs = {
        "x": rng.standard_normal((B, N, C), dtype=np.float32),
        "w_qkv": (rng.standard_normal((C, 3 * C)) * 0.02).astype(np.float32),
        "b_qkv": np.zeros((3 * C,), np.float32),
        "w_proj": (rng.standard_normal((C, C)) * 0.02).astype(np.float32),
        "b_proj": np.zeros((C,), np.float32),
    }
    got = kernel(**inputs)
    want = _numpy_fallback(**inputs)
    err = np.linalg.norm(got - want) / np.linalg.norm(want)
    print("rel l2 err vs numpy:", err)
